# revision 35
# baseline (speedup 1.0000x reference)
"""Multi-head attention Bass/Tile kernel for Trainium2.

Problem: nn_MultiHeadAttention  (B=8, S=1024, D=768, H=12, HD=64)
  q = x_h @ Wq^T + bq ; k,v likewise (per head)
  scores = q @ k^T        (NO pre-softmax scaling)
  attn = softmax(scores, -1) / sqrt(64)
  out = attn @ v, heads concatenated -> [B, S, D]

Sharding: data parallel over batch, one sample per NeuronCore (8 cores).
~136 us/core (from the 154 us fp32r baseline, ~300 us naive).

Key design points:
- Transposed layouts: scoresT[t, s] per t-chunk so softmax reduces along
  the free axis; exp with no max-subtraction (scores ~ N(0,64), fp32/bf16
  range safe); row-sums via an 8.0-scaled ones column in the V weights
  (folds the post-softmax /sqrt(64)); biases fold via a ones row in x^T;
  K zero-padded to 128 (half-height matmuls throttle the PE clock, and
  65/66-partition DMAs returned wrong data on HW).
- The q/k datapath runs in fp16 (x, Wqk, qkt, zq): 1 cycle/row, 1-pass
  LDWEIGHTS, half the DMA bytes of fp32r, and ~5e-3 logit error - well
  inside the 2e-2 gate.  attn@V runs in bf16.
- scoresT matmul: lhsT = [Q^T; K^T] stacked chunks (qkt), moving = [0;
  Q^T] (zq).  zq is not a second projection: the Q^T rows of qkt are
  DMA-copied into pre-zeroed rotating zq buffers (head 0 still projects
  via wzq0 weights - a DMA round-trip would sit on the prologue critical
  path).
- exp is split across engines so no single engine paces the rounds: ACT
  does cols [0:CA) as true exp (bf16 out), DVE does cols [CA:S) with a
  one-instruction Schraudolph approximation - uint16(x*(128/ln2) + 16252)
  bitcast to bf16, ~3% sawtooth rel err on those columns (~1.1e-2
  end-to-end).
- attnv accumulates OT'[e', s] in two single-bank PSUM halves by s-column
  half: sh0 chunks in rounds 1-4 (then PSUM->SBUF copy on ACT mid-head),
  sh1 in rounds 5-7 + next iteration round 0 (copy on DVE), so each OT
  bank is long free before the next head's attnv needs it.
- Per round: scores pair, exp + Schraudolph, one aux step (proj h+2 /
  out h-2, finely chunked), attnv pair.
- Output: PE-transpose OT' (bf16) back to [s, e] in 128-chunks, divide by
  the rowsum column via DVE reciprocal+mul, flush column batches to HBM
  as heads complete (late heads flush inside out_steps; the last head's
  out path is drained in the same iteration as its attnv so the DMA tail
  stays short).
- Prologue: PE warmed up with dummy ident matmuls during the input DMA
  wait (the PE p-state ramps to full clock only after ~3 us of continuous
  work; any idle gap resets it).  Exp table preloaded via a throwaway
  activation.  Do NOT warm up on uninitialized SBUF: random-bit fp16
  matmuls tripped a sustained ~1.2x whole-core down-clock.
"""

import os
import sys

for _p in (
    "/opt/trn_rl_repo",
    "/root/.axon_site",
    "/root/.axon_site/_ro/trn_rl_repo",
    "/root/.axon_site/_ro/pypackages",
):
    if os.path.isdir(_p) and _p not in sys.path:
        sys.path.append(_p)

import numpy as np

import concourse.bacc as bacc
import concourse.bass as bass
import concourse.tile as tile
from concourse import mybir

B, S, D, H, HD = 8, 1024, 768, 12, 64
K1 = 128  # contraction rows for proj/V' (64 x + ones row + zero pad; partial-K loads broke on HW)
VW = 66  # V' chunk width (64 e + rowsum col + even pad)
NT = S // 128  # 8 t-chunks / s-chunks
F32 = mybir.dt.float32
F32R = mybir.dt.float32r
BF16 = mybir.dt.bfloat16
F16 = mybir.dt.float16
U16 = mybir.dt.uint16

CA = 768  # columns of each scoresT row-block exp'd on ACT; rest on DVE
SCH_A = 128.0 / float(np.log(2.0))  # Schraudolph bf16 scale
SCH_B = 16256.0 - 4.0  # 127<<7 minus calibration constant


def build_nc():
    nc = bacc.Bacc(
        "TRN2",
        target_bir_lowering=False,
        debug=False,
        num_devices=1,
    )

    xt_d = nc.dram_tensor("xt", [H, K1, S], F16, kind="ExternalInput").ap()
    wqk_d = nc.dram_tensor("wqk", [H, K1, 128], F16, kind="ExternalInput").ap()
    wzq0_d = nc.dram_tensor("wzq0", [K1, 128], F16, kind="ExternalInput").ap()
    wv_d = nc.dram_tensor("wv", [H, K1, VW], F16, kind="ExternalInput").ap()
    ident_d = nc.dram_tensor("ident", [128, 128], F32R, kind="ExternalInput").ap()
    y_d = nc.dram_tensor("y", [S, D], F32, kind="ExternalOutput").ap()

    from contextlib import ExitStack

    with tile.TileContext(nc) as tc:
        with ExitStack() as ctx:
            _emit(ctx, tc, xt_d, wqk_d, wzq0_d, wv_d, ident_d, y_d)

    nc.compile()
    return nc


def _emit(ctx, tc, xt_d, wqk_d, wzq0_d, wv_d, ident_d, y_d):
    nc = tc.nc
    Exp = mybir.ActivationFunctionType.Exp
    CopyFn = mybir.ActivationFunctionType.Copy
    Mult = mybir.AluOpType.mult
    Add = mybir.AluOpType.add

    consts = ctx.enter_context(tc.tile_pool(name="consts", bufs=1))
    qkt_pool = ctx.enter_context(tc.tile_pool(name="qkt", bufs=3))
    vp_pool = ctx.enter_context(tc.tile_pool(name="vp", bufs=4))
    attn_pool = ctx.enter_context(tc.tile_pool(name="attn", bufs=18))
    otsb_pool = ctx.enter_context(tc.tile_pool(name="otsb", bufs=2))
    recip_pool = ctx.enter_context(tc.tile_pool(name="recip", bufs=2))
    ps_sc = ctx.enter_context(tc.tile_pool(name="ps_sc", bufs=2, space="PSUM"))
    # OT' accumulates in two single-bank halves (s cols 0:512 / 512:1024)
    # so each bank frees right after its PSUM->SBUF copy
    ps_ota = ctx.enter_context(tc.tile_pool(name="ps_ota", bufs=1, space="PSUM"))
    ps_otb = ctx.enter_context(tc.tile_pool(name="ps_otb", bufs=1, space="PSUM"))
    ps_misc = ctx.enter_context(tc.tile_pool(name="ps_misc", bufs=2, space="PSUM"))

    # ---- constant loads -------------------------------------------------
    wqk_sb = consts.tile([K1, H, 128], F16, name="wqk_sb")
    wzq0_sb = consts.tile([K1, 128], F16, name="wzq0_sb")
    wv_sb = consts.tile([K1, H, VW], F16, name="wv_sb")

    wqk_p = wqk_d.rearrange("h p j -> p h j")
    wv_p = wv_d.rearrange("h p j -> p h j")

    xt_sb = []
    for h in range(H):
        xt_sb.append(consts.tile([K1, S], F16, name=f"xt{h}"))
    ident = consts.tile([128, 128], F32R, name="ident")
    ident_bf = consts.tile([128, 128], BF16, name="ident_bf")
    zq_tiles = [consts.tile([128, S], F16, name=f"zq{i}") for i in range(3)]
    ysb = consts.tile([128, NT, D], F32, name="ysb")

    # DMA queues are sync/scalar (HWDGE) + gpsimd (SWDGE).  xt0 + ident +
    # head-0 weights gate the prologue; spread them so each queue's serial
    # chain stays short.  ident goes first anywhere: it gates the PE
    # warmup that must start ~3us before the first real projection.
    # head-0 weights lead the sync queue: wzq0 gates the zq projection
    # that sits on the first-exp critical path
    nc.sync.dma_start(out=ident, in_=ident_d)
    nc.sync.dma_start(out=wqk_sb[:, 0:1, :], in_=wqk_p[:, 0:1, :])
    nc.sync.dma_start(out=wzq0_sb, in_=wzq0_d)
    for c in (6, 7):
        nc.sync.dma_start(
            out=xt_sb[0][:, 128 * c : 128 * c + 128],
            in_=xt_d[0][:, 128 * c : 128 * c + 128],
        )
    nc.sync.dma_start(out=wqk_sb[:, 1:2, :], in_=wqk_p[:, 1:2, :])
    for c in (0, 1, 2):
        nc.scalar.dma_start(
            out=xt_sb[0][:, 128 * c : 128 * c + 128],
            in_=xt_d[0][:, 128 * c : 128 * c + 128],
        )
    # Exp table preload: a throwaway activation so the 1.3us table load
    # runs during the DMA wait instead of before the first real exp
    scratch = recip_pool.tile([1, 2], F32, tag="rec")
    nc.scalar.activation(scratch, zq_tiles[1].bitcast(F32)[0:1, 0:2], Exp)
    nc.scalar.dma_start(out=wv_sb[:, 0:1, :], in_=wv_p[:, 0:1, :])
    # memsets first on gpsimd: they gate the exp-table preload
    nc.gpsimd.memset(zq_tiles[1].bitcast(F32)[0:64, :], 0.0)
    nc.gpsimd.memset(zq_tiles[2].bitcast(F32)[0:64, :], 0.0)
    for c in (3, 4, 5):
        nc.gpsimd.dma_start(
            out=xt_sb[0][:, 128 * c : 128 * c + 128],
            in_=xt_d[0][:, 128 * c : 128 * c + 128],
        )
    # xt1 + head-1 weights on sync (needed by proj(1) during head-0 rounds)
    for c in range(4):
        nc.sync.dma_start(
            out=xt_sb[1][:, 256 * c : 256 * c + 256],
            in_=xt_d[1][:, 256 * c : 256 * c + 256],
        )
    nc.sync.dma_start(out=wv_sb[:, 1:2, :], in_=wv_p[:, 1:2, :])
    # gpsimd: xt2/xt3 chunked, head-2 slices, bulk weights, remaining xt
    for h in (2, 3):
        for c in range(2):
            nc.gpsimd.dma_start(
                out=xt_sb[h][:, 512 * c : 512 * c + 512],
                in_=xt_d[h][:, 512 * c : 512 * c + 512],
            )
    nc.gpsimd.dma_start(out=wqk_sb[:, 2:3, :], in_=wqk_p[:, 2:3, :])
    nc.gpsimd.dma_start(out=wv_sb[:, 2:3, :], in_=wv_p[:, 2:3, :])
    nc.gpsimd.dma_start(out=wqk_sb[:, 3:H, :], in_=wqk_p[:, 3:H, :])
    nc.gpsimd.dma_start(out=wv_sb[:, 3:H, :], in_=wv_p[:, 3:H, :])
    for h in range(4, H):
        nc.gpsimd.dma_start(out=xt_sb[h], in_=xt_d[h])

    # PE warmup: dummy matmuls on ident keep the PE busy through the DMA
    # wait so the p-state ramp (full clock after 3us of continuous work)
    # completes before the first real projection
    nc.vector.tensor_copy(ident_bf, ident)
    warm_ps = ps_misc.tile([128, 512], F32, tag="misc")
    for _ in range(7):
        nc.tensor.matmul(warm_ps[:, 0:128], ident, ident, start=True, stop=True)

    # chunk-major view of y: yv[p, s_chunk, d] = y[128*s_chunk + p, d]
    yv = y_d.rearrange("(s p) d -> p s d", p=128)

    # ---- software pipeline over heads ----------------------------------
    at_tiles = {}  # h -> list of 8 attnT sbuf tiles
    vp_sb = {}  # h -> V' sbuf tile [128, 8*66 + pad]
    ot_sb = {}  # h -> OT' sbuf tile [98, 1024]
    proj_state = {}  # h -> qkt sbuf tile

    def vp_steps(h):
        # V' per t-chunk: [128, 66]; 4 chunks per 1-bank psum tile.
        # vp tail-padded so lhsT slices [66c : 66c+128] stay in-bounds.
        vp = vp_pool.tile([128, NT * VW + 64], BF16, tag="vp")
        vp_sb[h] = vp
        nc.vector.memset(vp[:, NT * VW : NT * VW + 64], 0.0)
        for quarter in range(4):
            vp_ps = ps_misc.tile([128, 2 * VW], F32, tag="misc")
            for i in range(2):
                tcn = 2 * quarter + i
                nc.tensor.matmul(
                    vp_ps[:, VW * i : VW * i + VW],
                    xt_sb[h][:, 128 * tcn : 128 * tcn + 128],
                    wv_sb[:, h, :],
                    start=True,
                    stop=True,
                )
            nc.vector.tensor_copy(
                vp[:, 2 * VW * quarter : 2 * VW * quarter + 2 * VW], vp_ps
            )
            yield

    def proj0_steps():
        # Head 0: qkt chunk 0 first (scores tcn 0/1 lhsT), then the full
        # zq (moving operand of every round) via wzq0, then the rest.
        # zq copies ride ACT (idle until the first exp) so the DVE CAST
        # chain is off the first-exp critical path.
        qkt = qkt_pool.tile([128, S], F16, tag="qkt")
        proj_state[0] = qkt
        zqt = zq_tiles[0]
        order = [("q", 0, 256), ("z", 0, 512), ("z", 1, 512)]
        order += [("q", c, 256) for c in range(1, 4)]
        for kind, idx, nw in order:
            p_ps = ps_misc.tile([128, nw], F32, tag="misc")
            w = wqk_sb[:, 0, :] if kind == "q" else wzq0_sb
            nc.tensor.matmul(
                p_ps,
                w,
                xt_sb[0][:, nw * idx : nw * idx + nw],
                start=True,
                stop=True,
            )
            if kind == "q":
                nc.vector.tensor_copy(qkt[:, nw * idx : nw * idx + nw], p_ps)
            elif idx == 0:
                nc.scalar.activation(
                    zqt[:, nw * idx : nw * idx + nw], p_ps, CopyFn
                )
            else:
                # second zq half on DVE so both copies run in parallel
                nc.vector.tensor_copy(zqt[:, nw * idx : nw * idx + nw], p_ps)
            yield
        yield from vp_steps(0)

    def proj_steps(h):
        """Yield small chunks of head-h projection work (qkt + V'), to be
        sprinkled between the interleaved rounds.  zq comes from a DMA
        copy of the Q^T rows of qkt into a pre-zeroed rotating buffer."""
        qkt = qkt_pool.tile([128, S], F16, tag="qkt")
        proj_state[h] = qkt
        for sh in range(2):
            p_ps = ps_misc.tile([128, 512], F32, tag="misc")
            nc.tensor.matmul(
                p_ps,
                wqk_sb[:, h, :],
                xt_sb[h][:, 512 * sh : 512 * sh + 512],
                start=True,
                stop=True,
            )
            nc.vector.tensor_copy(qkt[:, 512 * sh : 512 * sh + 512], p_ps)
        yield
        # zq copy rides a DMA queue; lands well before head h's rounds
        nc.sync.dma_start(out=zq_tiles[h % 3][64:128, :], in_=qkt[0:64, :])
        yield from vp_steps(h)

    pend = []  # deferred OT finisher state: (hp, ot_b)

    def finish_prev():
        # attnv sh1 chunks 6-7 of the previous head, then the ot_b
        # PSUM->SBUF copy on DVE.  (ot_a was copied mid-head on ACT.)
        if not pend:
            return
        hp2, ot_b = pend.pop()
        vp2 = vp_sb[hp2]
        for tcn in (6, 7):
            nc.tensor.matmul(
                ot_b[0:98, :],
                vp2[:, VW * tcn : VW * tcn + 98],
                at_tiles[hp2][tcn][:, 512:S],
                start=False,
                stop=(tcn == 7),
            )
        nc.vector.tensor_copy(ot_sb[hp2][:, 512:S], ot_b[0:98, :])
        del at_tiles[hp2]
        del vp_sb[hp2]

    def emit_sc_av(h, hp, nxt=None):
        # Per round: scores(h) pair (frees exp ASAP), one aux step, then
        # attnv(hp).  exp is split ACT/DVE; attnv accumulates OT' by
        # column half: sh0 (ot_a) chunks 0-7 in rounds 1-4, sh1 (ot_b)
        # chunks 0-5 in rounds 5-7, chunks 6-7 next iteration.  Each OT
        # bank's copy then runs far from the moment the next head needs
        # the bank, so the hand-off never stalls the PE, and the ACT copy
        # (ot_a, after round 4) sits mid-head instead of at the exp
        # boundary.
        ats = []
        ot_a = ot_b = vp = ot = None
        if hp is not None:
            vp = vp_sb[hp]
        for tcn in range(NT):
            if tcn == 0:
                finish_prev()
                if hp is not None:
                    ot_a = ps_ota.tile([128, 512], F32, tag="ota")
                    ot_b = ps_otb.tile([128, 512], F32, tag="otb")
                    ot = otsb_pool.tile([98, S], BF16, tag="ot_sb")
                    ot_sb[hp] = ot
            if h is not None:
                qkt = proj_state[h]
                zqt = zq_tiles[h % 3]
                sc_ps = ps_sc.tile([128, S], F32, tag="sc")
                lhsT = qkt[:, 128 * tcn : 128 * tcn + 128]  # [Q^T; K^T] chunk
                for sh in range(2):
                    nc.tensor.matmul(
                        sc_ps[:, 512 * sh : 512 * sh + 512],
                        lhsT,
                        zqt[:, 512 * sh : 512 * sh + 512],
                        start=True,
                        stop=True,
                    )
                at = attn_pool.tile([128, S], BF16, tag="at")
                nc.scalar.activation(at[:, 0:CA], sc_ps[:, 0:CA], Exp)
                nc.vector.tensor_scalar(
                    at.bitcast(U16)[:, CA:S],
                    sc_ps[:, CA:S],
                    SCH_A,
                    SCH_B,
                    Mult,
                    Add,
                )
                ats.append(at)
            if nxt is not None:
                next(nxt, None)
                if tcn == 0:
                    next(nxt, None)
            if hp is not None and tcn > 0:
                if tcn <= 4:
                    sh, ot_h, chunks = 0, ot_a, (2 * tcn - 2, 2 * tcn - 1)
                else:
                    sh, ot_h, chunks = 1, ot_b, (2 * tcn - 10, 2 * tcn - 9)
                for ck in chunks:
                    # M=98: near-smallest col count spanning all four PE
                    # column groups
                    nc.tensor.matmul(
                        ot_h[0:98, :],
                        vp[:, VW * ck : VW * ck + 98],
                        at_tiles[hp][ck][:, 512 * sh : 512 * sh + 512],
                        start=(tcn == 1 or tcn == 5) and ck == chunks[0],
                        stop=sh == 0 and ck == 7,
                    )
                if tcn == 4:
                    # sh0 complete: copy ot_a to SBUF on ACT mid-head
                    nc.scalar.activation(ot[:, 0:512], ot_a[0:98, :], CopyFn)
        if h is not None:
            at_tiles[h] = ats
        if hp is not None:
            pend.append((hp, ot_b))

    def out_steps(h):
        # PE-transpose OT' back to [s, e] in 128-chunks; col 64 = 8*rowsum.
        # 4 fine yields (2 transposes each) so no single round carries the
        # whole transpose block.  Late heads flush their own column batch
        # per half so the final DMA tail stays short.
        ot = ot_sb[h]
        for half in range(2):
            ott_ps = ps_misc.tile([128, 4 * 128], BF16, tag="misc")
            for i in (0, 1):
                scn = 4 * half + i
                nc.tensor.transpose(
                    ott_ps[:, 128 * i : 128 * i + 98],
                    ot[:, 128 * scn : 128 * scn + 128],
                    ident_bf[0:98, 0:98],
                )
            yield
            for i in (2, 3):
                scn = 4 * half + i
                nc.tensor.transpose(
                    ott_ps[:, 128 * i : 128 * i + 98],
                    ot[:, 128 * scn : 128 * scn + 128],
                    ident_bf[0:98, 0:98],
                )
            ottv = ott_ps.rearrange("p (c w) -> p c w", w=128)
            rec = recip_pool.tile([128, 4], F32, tag="rec")
            nc.vector.reciprocal(rec, ottv[:, :, 64])
            rec_b = bass.AP(
                tensor=rec.tensor, offset=rec.offset, ap=list(rec.ap) + [[0, 64]]
            )
            nc.vector.tensor_mul(
                ysb[:, 4 * half : 4 * half + 4, 64 * h : 64 * h + 64],
                ottv[:, :, 0:64],
                rec_b,
            )
            if h >= 9:
                # flush this head's columns for the finished s-chunks in
                # one DMA via the chunk-major view of y (scalar queue only
                # for the last head - it would delay exps)
                eng = (nc.scalar if h == H - 1 else nc.gpsimd) if half else nc.sync
                eng.dma_start(
                    out=yv[:, 4 * half : 4 * half + 4, 64 * h : 64 * h + 64],
                    in_=ysb[:, 4 * half : 4 * half + 4, 64 * h : 64 * h + 64],
                )
            yield
        del ot_sb[h]

    import itertools

    g0 = proj0_steps()
    for _ in range(5):  # qkt c0-c2 + both zq halves: rounds 0-5 covered
        next(g0)
    for h in range(H + 1):
        cur = h if h < H else None
        prev = h - 1 if 1 <= h <= H else None
        gens = []
        # head 0 finishes its own projection + proj(1); head 1 (whose
        # rounds have no out() work yet) carries proj(2) and proj(3)
        if h == 0:
            gens += [g0, proj_steps(1)]
        elif h == 1:
            gens += [proj_steps(2), proj_steps(3)]
        elif h + 2 < H:
            gens.append(proj_steps(h + 2))
        if 2 <= h and h - 2 < H:
            gens.append(out_steps(h - 2))
        nxt = itertools.chain(*gens) if gens else None
        if cur is not None or prev is not None:
            emit_sc_av(cur, prev, nxt)
        if nxt is not None:
            for _ in nxt:  # drain any remaining steps
                pass
        if h == H:
            # last head (iteration 12, no exp pacing): finish its deferred
            # attnv chunks + OT copies and emit out(11) immediately so the
            # final flush isn't serialized behind an extra iteration
            finish_prev()
            for _ in out_steps(H - 1):
                pass
        if cur is not None:
            proj_state.pop(h)
        if h in (6, 10):
            # flush completed output columns while later heads compute,
            # shrinking the final DMA tail (out(h-2) done by iteration h;
            # heads 9-11 self-flush inside out_steps)
            c0, c1 = {6: (0, 320), 10: (320, 576)}[h]
            for half in range(2):
                eng = nc.sync if half == 0 else nc.gpsimd
                eng.dma_start(
                    out=yv[:, 4 * half : 4 * half + 4, c0:c1],
                    in_=ysb[:, 4 * half : 4 * half + 4, c0:c1],
                )


# --------------------------------------------------------------------------
# host side
# --------------------------------------------------------------------------

_NC_CACHE = {}

LAST_EXEC_NS = None
LAST_RESULTS = None


def _get_nc():
    if "nc" not in _NC_CACHE:
        _NC_CACHE["nc"] = build_nc()
    return _NC_CACHE["nc"]


def prep_inputs(x, Wq, bq, Wk, bk, Wv, bv):
    """Host-side layout prep. Returns per-core input maps."""
    x = np.ascontiguousarray(np.asarray(x, dtype=np.float32))
    Wq, bq = np.asarray(Wq, np.float32), np.asarray(bq, np.float32)
    Wk, bk = np.asarray(Wk, np.float32), np.asarray(bk, np.float32)
    Wv, bv = np.asarray(Wv, np.float32), np.asarray(bv, np.float32)

    # xt: [B, H, 128, S]: rows 0-63 = x^T, row 64 = ones, rows 65-127 = 0
    # (zero-padded to K=128 so every matmul keeps the full PE array active —
    #  half-height matmuls trip the HAM activity monitor into throttling)
    xt = np.zeros((B, H, K1, S), np.float16)
    xt[:, :, :HD] = x.transpose(0, 2, 1).reshape(B, H, HD, S)
    xt[:, :, HD] = 1.0

    def stack2(Wa, ba, Wb, bb):
        w = np.zeros((H, K1, 128), np.float32)
        w[:, :HD, :HD] = Wa.transpose(0, 2, 1)
        w[:, :HD, HD:] = Wb.transpose(0, 2, 1)
        w[:, HD, :HD] = ba
        w[:, HD, HD:] = bb
        return w

    wqk = stack2(Wq, bq, Wk, bk).astype(np.float16)
    wzq0 = np.zeros((K1, 128), np.float32)
    wzq0[:HD, HD:] = Wq[0].T
    wzq0[HD, HD:] = bq[0]

    wv = np.zeros((H, K1, VW), np.float32)
    wv[:, :HD, :HD] = Wv.transpose(0, 2, 1)
    wv[:, HD, :HD] = bv
    wv[:, HD, HD] = 8.0  # ones col scaled by sqrt(HD) -> folds post-softmax /8

    wzq0 = wzq0.astype(np.float16)
    wv = wv.astype(np.float16)
    ident = np.eye(128, dtype=np.float32)

    return [
        {"xt": xt[b], "wqk": wqk, "wzq0": wzq0, "wv": wv, "ident": ident}
        for b in range(B)
    ]


def kernel(x, Wq, bq, Wk, bk, Wv, bv):
    global LAST_EXEC_NS, LAST_RESULTS
    from concourse.bass_utils import run_bass_kernel_spmd

    nc = _get_nc()
    in_maps = prep_inputs(x, Wq, bq, Wk, bk, Wv, bv)
    trace = os.environ.get("KERNEL_TRACE", "0") == "1"
    res = run_bass_kernel_spmd(
        nc,
        in_maps,
        core_ids=list(range(B)),
        trace=trace,
    )
    LAST_EXEC_NS = res.exec_time_ns
    LAST_RESULTS = res
    y = np.stack([res.results[b]["y"] for b in range(B)], axis=0)
    return y.astype(np.float32)


# revision 36
# speedup vs baseline: 1.1868x; 1.1868x over previous
"""Multi-head attention Bass/Tile kernel for Trainium2.

Problem: nn_MultiHeadAttention  (B=8, S=1024, D=768, H=12, HD=64)
  q = x_h @ Wq^T + bq ; k,v likewise (per head)
  scores = q @ k^T        (NO pre-softmax scaling)
  attn = softmax(scores, -1) / sqrt(64)
  out = attn @ v, heads concatenated -> [B, S, D]

Sharding: data parallel over batch, one sample per NeuronCore (8 cores).
~136 us/core (from the 154 us fp32r baseline, ~300 us naive).

Key design points:
- Transposed layouts: scoresT[t, s] per t-chunk so softmax reduces along
  the free axis; exp with no max-subtraction (scores ~ N(0,64), fp32/bf16
  range safe); row-sums via an 8.0-scaled ones column in the V weights
  (folds the post-softmax /sqrt(64)); biases fold via a ones row in x^T;
  K zero-padded to 128 (half-height matmuls throttle the PE clock, and
  65/66-partition DMAs returned wrong data on HW).
- The q/k datapath runs in fp16 (x, Wqk, qkt, zq): 1 cycle/row, 1-pass
  LDWEIGHTS, half the DMA bytes of fp32r, and ~5e-3 logit error - well
  inside the 2e-2 gate.  attn@V runs in bf16.
- scoresT matmul: lhsT = [Q^T; K^T] stacked chunks (qkt), moving = [0;
  Q^T] (zq).  zq is not a second projection: the Q^T rows of qkt are
  DMA-copied into pre-zeroed rotating zq buffers (head 0 still projects
  via wzq0 weights - a DMA round-trip would sit on the prologue critical
  path).
- exp is split across engines so no single engine paces the rounds: ACT
  does cols [0:CA) as true exp (bf16 out), DVE does cols [CA:S) with a
  one-instruction Schraudolph approximation - uint16(x*(128/ln2) + 16252)
  bitcast to bf16, ~3% sawtooth rel err on those columns (~1.1e-2
  end-to-end).
- attnv accumulates OT'[e', s] in two single-bank PSUM halves by s-column
  half: sh0 chunks in rounds 1-4 (then PSUM->SBUF copy on ACT mid-head),
  sh1 in rounds 5-7 + next iteration round 0 (copy on DVE), so each OT
  bank is long free before the next head's attnv needs it.
- Per round: scores pair, exp + Schraudolph, one aux step (proj h+2 /
  out h-2, finely chunked), attnv pair.
- Output: PE-transpose OT' (bf16) back to [s, e] in 128-chunks, divide by
  the rowsum column via DVE reciprocal+mul, flush column batches to HBM
  as heads complete (late heads flush inside out_steps; the last head's
  out path is drained in the same iteration as its attnv so the DMA tail
  stays short).
- Prologue: PE warmed up with dummy ident matmuls during the input DMA
  wait (the PE p-state ramps to full clock only after ~3 us of continuous
  work; any idle gap resets it).  Exp table preloaded via a throwaway
  activation.  Do NOT warm up on uninitialized SBUF: random-bit fp16
  matmuls tripped a sustained ~1.2x whole-core down-clock.
"""

import os
import sys

for _p in (
    "/opt/trn_rl_repo",
    "/root/.axon_site",
    "/root/.axon_site/_ro/trn_rl_repo",
    "/root/.axon_site/_ro/pypackages",
):
    if os.path.isdir(_p) and _p not in sys.path:
        sys.path.append(_p)

import numpy as np

import concourse.bacc as bacc
import concourse.bass as bass
import concourse.tile as tile
from concourse import mybir

B, S, D, H, HD = 8, 1024, 768, 12, 64
K1 = 128  # contraction rows for proj/V' (64 x + ones row + zero pad; partial-K loads broke on HW)
VW = 66  # V' chunk width (64 e + rowsum col + even pad)
NT = S // 128  # 8 t-chunks / s-chunks
F32 = mybir.dt.float32
F32R = mybir.dt.float32r
BF16 = mybir.dt.bfloat16
F16 = mybir.dt.float16
U16 = mybir.dt.uint16

CA = 768  # columns of each scoresT row-block exp'd on ACT; rest on DVE
SCH_A = 128.0 / float(np.log(2.0))  # Schraudolph bf16 scale
SCH_B = 16256.0 - 4.0  # 127<<7 minus calibration constant


def build_nc():
    nc = bacc.Bacc(
        "TRN2",
        target_bir_lowering=False,
        debug=False,
        num_devices=1,
    )

    xt_d = nc.dram_tensor("xt", [H, K1, S], F16, kind="ExternalInput").ap()
    wqk_d = nc.dram_tensor("wqk", [H, K1, 128], F16, kind="ExternalInput").ap()
    wzq0_d = nc.dram_tensor("wzq0", [K1, 128], F16, kind="ExternalInput").ap()
    wv_d = nc.dram_tensor("wv", [H, K1, VW], F16, kind="ExternalInput").ap()
    ident_d = nc.dram_tensor("ident", [128, 128], F32R, kind="ExternalInput").ap()
    y_d = nc.dram_tensor("y", [S, D], F32, kind="ExternalOutput").ap()

    from contextlib import ExitStack

    with tile.TileContext(nc) as tc:
        with ExitStack() as ctx:
            _emit(ctx, tc, xt_d, wqk_d, wzq0_d, wv_d, ident_d, y_d)

    nc.compile()
    return nc


def _emit(ctx, tc, xt_d, wqk_d, wzq0_d, wv_d, ident_d, y_d):
    nc = tc.nc
    Exp = mybir.ActivationFunctionType.Exp
    CopyFn = mybir.ActivationFunctionType.Copy
    Mult = mybir.AluOpType.mult
    Add = mybir.AluOpType.add

    consts = ctx.enter_context(tc.tile_pool(name="consts", bufs=1))
    qkt_pool = ctx.enter_context(tc.tile_pool(name="qkt", bufs=3))
    vp_pool = ctx.enter_context(tc.tile_pool(name="vp", bufs=4))
    attn_pool = ctx.enter_context(tc.tile_pool(name="attn", bufs=18))
    otsb_pool = ctx.enter_context(tc.tile_pool(name="otsb", bufs=2))
    recip_pool = ctx.enter_context(tc.tile_pool(name="recip", bufs=2))
    ps_sc = ctx.enter_context(tc.tile_pool(name="ps_sc", bufs=2, space="PSUM"))
    # OT' accumulates in two single-bank halves (s cols 0:512 / 512:1024)
    # so each bank frees right after its PSUM->SBUF copy
    ps_ota = ctx.enter_context(tc.tile_pool(name="ps_ota", bufs=1, space="PSUM"))
    ps_otb = ctx.enter_context(tc.tile_pool(name="ps_otb", bufs=1, space="PSUM"))
    ps_misc = ctx.enter_context(tc.tile_pool(name="ps_misc", bufs=2, space="PSUM"))

    # ---- constant loads -------------------------------------------------
    wqk_sb = consts.tile([K1, H, 128], F16, name="wqk_sb")
    wzq0_sb = consts.tile([K1, 128], F16, name="wzq0_sb")
    wv_sb = consts.tile([K1, H, VW], F16, name="wv_sb")

    wqk_p = wqk_d.rearrange("h p j -> p h j")
    wv_p = wv_d.rearrange("h p j -> p h j")

    xt_sb = []
    for h in range(H):
        xt_sb.append(consts.tile([K1, S], F16, name=f"xt{h}"))
    ident = consts.tile([128, 128], F32R, name="ident")
    ident_bf = consts.tile([128, 128], BF16, name="ident_bf")
    zq_tiles = [consts.tile([128, S], F16, name=f"zq{i}") for i in range(3)]
    ysb = consts.tile([128, NT, D], F32, name="ysb")

    # DMA queues are sync/scalar (HWDGE) + gpsimd (SWDGE).  xt0 + ident +
    # head-0 weights gate the prologue; spread them so each queue's serial
    # chain stays short.  ident goes first anywhere: it gates the PE
    # warmup that must start ~3us before the first real projection.
    # head-0 weights lead the sync queue: wzq0 gates the zq projection
    # that sits on the first-exp critical path
    nc.sync.dma_start(out=ident, in_=ident_d)
    nc.sync.dma_start(out=wqk_sb[:, 0:1, :], in_=wqk_p[:, 0:1, :])
    nc.sync.dma_start(out=wzq0_sb, in_=wzq0_d)
    for c in (6, 7):
        nc.sync.dma_start(
            out=xt_sb[0][:, 128 * c : 128 * c + 128],
            in_=xt_d[0][:, 128 * c : 128 * c + 128],
        )
    nc.sync.dma_start(out=wqk_sb[:, 1:2, :], in_=wqk_p[:, 1:2, :])
    for c in (0, 1, 2):
        nc.scalar.dma_start(
            out=xt_sb[0][:, 128 * c : 128 * c + 128],
            in_=xt_d[0][:, 128 * c : 128 * c + 128],
        )
    # Exp table preload: a throwaway activation so the 1.3us table load
    # runs during the DMA wait instead of before the first real exp
    scratch = recip_pool.tile([1, 2], F32, tag="rec")
    nc.scalar.activation(scratch, zq_tiles[1].bitcast(F32)[0:1, 0:2], Exp)
    nc.scalar.dma_start(out=wv_sb[:, 0:1, :], in_=wv_p[:, 0:1, :])
    # memsets first on gpsimd: they gate the exp-table preload
    nc.gpsimd.memset(zq_tiles[1].bitcast(F32)[0:64, :], 0.0)
    nc.gpsimd.memset(zq_tiles[2].bitcast(F32)[0:64, :], 0.0)
    for c in (3, 4, 5):
        nc.gpsimd.dma_start(
            out=xt_sb[0][:, 128 * c : 128 * c + 128],
            in_=xt_d[0][:, 128 * c : 128 * c + 128],
        )
    # xt1 + head-1 weights on sync (needed by proj(1) during head-0 rounds)
    for c in range(4):
        nc.sync.dma_start(
            out=xt_sb[1][:, 256 * c : 256 * c + 256],
            in_=xt_d[1][:, 256 * c : 256 * c + 256],
        )
    nc.sync.dma_start(out=wv_sb[:, 1:2, :], in_=wv_p[:, 1:2, :])
    # gpsimd: xt2/xt3 chunked, head-2 slices, bulk weights, remaining xt
    for h in (2, 3):
        for c in range(2):
            nc.gpsimd.dma_start(
                out=xt_sb[h][:, 512 * c : 512 * c + 512],
                in_=xt_d[h][:, 512 * c : 512 * c + 512],
            )
    nc.gpsimd.dma_start(out=wqk_sb[:, 2:3, :], in_=wqk_p[:, 2:3, :])
    nc.gpsimd.dma_start(out=wv_sb[:, 2:3, :], in_=wv_p[:, 2:3, :])
    nc.gpsimd.dma_start(out=wqk_sb[:, 3:H, :], in_=wqk_p[:, 3:H, :])
    nc.gpsimd.dma_start(out=wv_sb[:, 3:H, :], in_=wv_p[:, 3:H, :])
    for h in range(4, H):
        nc.gpsimd.dma_start(out=xt_sb[h], in_=xt_d[h])

    # PE warmup: dummy matmuls on ident keep the PE busy through the DMA
    # wait so the p-state ramp (full clock after 3us of continuous work)
    # completes before the first real projection
    nc.vector.tensor_copy(ident_bf, ident)
    warm_ps = ps_misc.tile([128, 512], F32, tag="misc")
    for _ in range(7):
        nc.tensor.matmul(warm_ps[:, 0:128], ident, ident, start=True, stop=True)

    # chunk-major view of y: yv[p, s_chunk, d] = y[128*s_chunk + p, d]
    yv = y_d.rearrange("(s p) d -> p s d", p=128)

    # ---- software pipeline over heads ----------------------------------
    at_tiles = {}  # h -> list of 8 attnT sbuf tiles
    vp_sb = {}  # h -> V' sbuf tile [128, 8*66 + pad]
    ot_sb = {}  # h -> OT' sbuf tile [98, 1024]
    proj_state = {}  # h -> qkt sbuf tile

    def vp_steps(h):
        # V' per t-chunk: [128, 66]; 4 chunks per 1-bank psum tile.
        # vp tail-padded so lhsT slices [66c : 66c+128] stay in-bounds.
        vp = vp_pool.tile([128, NT * VW + 64], BF16, tag="vp")
        vp_sb[h] = vp
        nc.vector.memset(vp[:, NT * VW : NT * VW + 64], 0.0)
        for quarter in range(4):
            vp_ps = ps_misc.tile([128, 2 * VW], F32, tag="misc")
            for i in range(2):
                tcn = 2 * quarter + i
                nc.tensor.matmul(
                    vp_ps[:, VW * i : VW * i + VW],
                    xt_sb[h][:, 128 * tcn : 128 * tcn + 128],
                    wv_sb[:, h, :],
                    start=True,
                    stop=True,
                )
            nc.vector.tensor_copy(
                vp[:, 2 * VW * quarter : 2 * VW * quarter + 2 * VW], vp_ps
            )
            yield

    def proj0_steps():
        # Head 0: qkt chunk 0 first (scores tcn 0/1 lhsT), then the full
        # zq (moving operand of every round) via wzq0, then the rest.
        # zq copies ride ACT (idle until the first exp) so the DVE CAST
        # chain is off the first-exp critical path.
        qkt = qkt_pool.tile([128, S], F16, tag="qkt")
        proj_state[0] = qkt
        zqt = zq_tiles[0]
        order = [("q", 0, 256), ("z", 0, 512), ("z", 1, 512)]
        order += [("q", c, 256) for c in range(1, 4)]
        for kind, idx, nw in order:
            p_ps = ps_misc.tile([128, nw], F32, tag="misc")
            w = wqk_sb[:, 0, :] if kind == "q" else wzq0_sb
            nc.tensor.matmul(
                p_ps,
                w,
                xt_sb[0][:, nw * idx : nw * idx + nw],
                start=True,
                stop=True,
            )
            if kind == "q":
                nc.vector.tensor_copy(qkt[:, nw * idx : nw * idx + nw], p_ps)
            else:
                nc.scalar.activation(
                    zqt[:, nw * idx : nw * idx + nw], p_ps, CopyFn
                )
            yield
        yield from vp_steps(0)

    def proj_steps(h):
        """Yield small chunks of head-h projection work (qkt + V'), to be
        sprinkled between the interleaved rounds.  zq comes from a DMA
        copy of the Q^T rows of qkt into a pre-zeroed rotating buffer."""
        qkt = qkt_pool.tile([128, S], F16, tag="qkt")
        proj_state[h] = qkt
        for sh in range(2):
            p_ps = ps_misc.tile([128, 512], F32, tag="misc")
            nc.tensor.matmul(
                p_ps,
                wqk_sb[:, h, :],
                xt_sb[h][:, 512 * sh : 512 * sh + 512],
                start=True,
                stop=True,
            )
            nc.vector.tensor_copy(qkt[:, 512 * sh : 512 * sh + 512], p_ps)
        yield
        # zq copy rides a DMA queue; lands well before head h's rounds
        nc.sync.dma_start(out=zq_tiles[h % 3][64:128, :], in_=qkt[0:64, :])
        yield from vp_steps(h)

    pend = []  # deferred OT finisher state: (hp, ot_b)

    def finish_prev():
        # attnv sh1 chunks 6-7 of the previous head, then the ot_b
        # PSUM->SBUF copy on DVE.  (ot_a was copied mid-head on ACT.)
        if not pend:
            return
        hp2, ot_b = pend.pop()
        vp2 = vp_sb[hp2]
        for tcn in (6, 7):
            nc.tensor.matmul(
                ot_b[0:98, :],
                vp2[:, VW * tcn : VW * tcn + 98],
                at_tiles[hp2][tcn][:, 512:S],
                start=False,
                stop=(tcn == 7),
            )
        nc.vector.tensor_copy(ot_sb[hp2][:, 512:S], ot_b[0:98, :])
        del at_tiles[hp2]
        del vp_sb[hp2]

    def emit_sc_av(h, hp, nxt=None):
        # Per round: scores(h) pair (frees exp ASAP), one aux step, then
        # attnv(hp).  exp is split ACT/DVE; attnv accumulates OT' by
        # column half: sh0 (ot_a) chunks 0-7 in rounds 1-4, sh1 (ot_b)
        # chunks 0-5 in rounds 5-7, chunks 6-7 next iteration.  Each OT
        # bank's copy then runs far from the moment the next head needs
        # the bank, so the hand-off never stalls the PE, and the ACT copy
        # (ot_a, after round 4) sits mid-head instead of at the exp
        # boundary.
        ats = []
        ot_a = ot_b = vp = ot = None
        if hp is not None:
            vp = vp_sb[hp]
        for tcn in range(NT):
            if tcn == 0:
                finish_prev()
                if hp is not None:
                    ot_a = ps_ota.tile([128, 512], F32, tag="ota")
                    ot_b = ps_otb.tile([128, 512], F32, tag="otb")
                    ot = otsb_pool.tile([98, S], BF16, tag="ot_sb")
                    ot_sb[hp] = ot
            if h is not None:
                qkt = proj_state[h]
                zqt = zq_tiles[h % 3]
                sc_ps = ps_sc.tile([128, S], F32, tag="sc")
                lhsT = qkt[:, 128 * tcn : 128 * tcn + 128]  # [Q^T; K^T] chunk
                for sh in range(2):
                    nc.tensor.matmul(
                        sc_ps[:, 512 * sh : 512 * sh + 512],
                        lhsT,
                        zqt[:, 512 * sh : 512 * sh + 512],
                        start=True,
                        stop=True,
                    )
                at = attn_pool.tile([128, S], BF16, tag="at")
                nc.scalar.activation(at[:, 0:CA], sc_ps[:, 0:CA], Exp)
                nc.vector.tensor_scalar(
                    at.bitcast(U16)[:, CA:S],
                    sc_ps[:, CA:S],
                    SCH_A,
                    SCH_B,
                    Mult,
                    Add,
                )
                ats.append(at)
            if nxt is not None:
                next(nxt, None)
                if tcn == 0:
                    next(nxt, None)
            if hp is not None and tcn > 0:
                if tcn <= 4:
                    sh, ot_h, chunks = 0, ot_a, (2 * tcn - 2, 2 * tcn - 1)
                else:
                    sh, ot_h, chunks = 1, ot_b, (2 * tcn - 10, 2 * tcn - 9)
                for ck in chunks:
                    # M=98: near-smallest col count spanning all four PE
                    # column groups
                    nc.tensor.matmul(
                        ot_h[0:98, :],
                        vp[:, VW * ck : VW * ck + 98],
                        at_tiles[hp][ck][:, 512 * sh : 512 * sh + 512],
                        start=(tcn == 1 or tcn == 5) and ck == chunks[0],
                        stop=sh == 0 and ck == 7,
                    )
                if tcn == 4:
                    # sh0 complete: copy ot_a to SBUF on ACT mid-head
                    nc.scalar.activation(ot[:, 0:512], ot_a[0:98, :], CopyFn)
        if h is not None:
            at_tiles[h] = ats
        if hp is not None:
            pend.append((hp, ot_b))

    def out_steps(h):
        # PE-transpose OT' back to [s, e] in 128-chunks; col 64 = 8*rowsum.
        # 4 fine yields (2 transposes each) so no single round carries the
        # whole transpose block.  Late heads flush their own column batch
        # per half so the final DMA tail stays short.
        ot = ot_sb[h]
        for half in range(2):
            ott_ps = ps_misc.tile([128, 4 * 128], BF16, tag="misc")
            for i in (0, 1):
                scn = 4 * half + i
                nc.tensor.transpose(
                    ott_ps[:, 128 * i : 128 * i + 98],
                    ot[:, 128 * scn : 128 * scn + 128],
                    ident_bf[0:98, 0:98],
                )
            yield
            for i in (2, 3):
                scn = 4 * half + i
                nc.tensor.transpose(
                    ott_ps[:, 128 * i : 128 * i + 98],
                    ot[:, 128 * scn : 128 * scn + 128],
                    ident_bf[0:98, 0:98],
                )
            ottv = ott_ps.rearrange("p (c w) -> p c w", w=128)
            rec = recip_pool.tile([128, 4], F32, tag="rec")
            nc.vector.reciprocal(rec, ottv[:, :, 64])
            rec_b = bass.AP(
                tensor=rec.tensor, offset=rec.offset, ap=list(rec.ap) + [[0, 64]]
            )
            nc.vector.tensor_mul(
                ysb[:, 4 * half : 4 * half + 4, 64 * h : 64 * h + 64],
                ottv[:, :, 0:64],
                rec_b,
            )
            if h >= 9:
                # flush this head's columns for the finished s-chunks in
                # one DMA via the chunk-major view of y (scalar queue only
                # for the last head - it would delay exps)
                eng = (nc.scalar if h == H - 1 else nc.gpsimd) if half else nc.sync
                eng.dma_start(
                    out=yv[:, 4 * half : 4 * half + 4, 64 * h : 64 * h + 64],
                    in_=ysb[:, 4 * half : 4 * half + 4, 64 * h : 64 * h + 64],
                )
            yield
        del ot_sb[h]

    import itertools

    g0 = proj0_steps()
    for _ in range(5):  # qkt c0-c2 + both zq halves: rounds 0-5 covered
        next(g0)
    for h in range(H + 1):
        cur = h if h < H else None
        prev = h - 1 if 1 <= h <= H else None
        gens = []
        # head 0 finishes its own projection + proj(1); head 1 (whose
        # rounds have no out() work yet) carries proj(2) and proj(3)
        if h == 0:
            gens += [g0, proj_steps(1)]
        elif h == 1:
            gens += [proj_steps(2), proj_steps(3)]
        elif h + 2 < H:
            gens.append(proj_steps(h + 2))
        if 2 <= h and h - 2 < H:
            gens.append(out_steps(h - 2))
        nxt = itertools.chain(*gens) if gens else None
        if cur is not None or prev is not None:
            emit_sc_av(cur, prev, nxt)
        if nxt is not None:
            for _ in nxt:  # drain any remaining steps
                pass
        if h == H:
            # last head (iteration 12, no exp pacing): finish its deferred
            # attnv chunks + OT copies and emit out(11) immediately so the
            # final flush isn't serialized behind an extra iteration
            finish_prev()
            for _ in out_steps(H - 1):
                pass
        if cur is not None:
            proj_state.pop(h)
        if h in (6, 10):
            # flush completed output columns while later heads compute,
            # shrinking the final DMA tail (out(h-2) done by iteration h;
            # heads 9-11 self-flush inside out_steps)
            c0, c1 = {6: (0, 320), 10: (320, 576)}[h]
            for half in range(2):
                eng = nc.sync if half == 0 else nc.gpsimd
                eng.dma_start(
                    out=yv[:, 4 * half : 4 * half + 4, c0:c1],
                    in_=ysb[:, 4 * half : 4 * half + 4, c0:c1],
                )


# --------------------------------------------------------------------------
# host side
# --------------------------------------------------------------------------

_NC_CACHE = {}

LAST_EXEC_NS = None
LAST_RESULTS = None


def _get_nc():
    if "nc" not in _NC_CACHE:
        _NC_CACHE["nc"] = build_nc()
    return _NC_CACHE["nc"]


def prep_inputs(x, Wq, bq, Wk, bk, Wv, bv):
    """Host-side layout prep. Returns per-core input maps."""
    x = np.ascontiguousarray(np.asarray(x, dtype=np.float32))
    Wq, bq = np.asarray(Wq, np.float32), np.asarray(bq, np.float32)
    Wk, bk = np.asarray(Wk, np.float32), np.asarray(bk, np.float32)
    Wv, bv = np.asarray(Wv, np.float32), np.asarray(bv, np.float32)

    # xt: [B, H, 128, S]: rows 0-63 = x^T, row 64 = ones, rows 65-127 = 0
    # (zero-padded to K=128 so every matmul keeps the full PE array active —
    #  half-height matmuls trip the HAM activity monitor into throttling)
    xt = np.zeros((B, H, K1, S), np.float16)
    xt[:, :, :HD] = x.transpose(0, 2, 1).reshape(B, H, HD, S)
    xt[:, :, HD] = 1.0

    def stack2(Wa, ba, Wb, bb):
        w = np.zeros((H, K1, 128), np.float32)
        w[:, :HD, :HD] = Wa.transpose(0, 2, 1)
        w[:, :HD, HD:] = Wb.transpose(0, 2, 1)
        w[:, HD, :HD] = ba
        w[:, HD, HD:] = bb
        return w

    wqk = stack2(Wq, bq, Wk, bk).astype(np.float16)
    wzq0 = np.zeros((K1, 128), np.float32)
    wzq0[:HD, HD:] = Wq[0].T
    wzq0[HD, HD:] = bq[0]

    wv = np.zeros((H, K1, VW), np.float32)
    wv[:, :HD, :HD] = Wv.transpose(0, 2, 1)
    wv[:, HD, :HD] = bv
    wv[:, HD, HD] = 8.0  # ones col scaled by sqrt(HD) -> folds post-softmax /8

    wzq0 = wzq0.astype(np.float16)
    wv = wv.astype(np.float16)
    ident = np.eye(128, dtype=np.float32)

    return [
        {"xt": xt[b], "wqk": wqk, "wzq0": wzq0, "wv": wv, "ident": ident}
        for b in range(B)
    ]


def kernel(x, Wq, bq, Wk, bk, Wv, bv):
    global LAST_EXEC_NS, LAST_RESULTS
    from concourse.bass_utils import run_bass_kernel_spmd

    nc = _get_nc()
    in_maps = prep_inputs(x, Wq, bq, Wk, bk, Wv, bv)
    trace = os.environ.get("KERNEL_TRACE", "0") == "1"
    res = run_bass_kernel_spmd(
        nc,
        in_maps,
        core_ids=list(range(B)),
        trace=trace,
    )
    LAST_EXEC_NS = res.exec_time_ns
    LAST_RESULTS = res
    y = np.stack([res.results[b]["y"] for b in range(B)], axis=0)
    return y.astype(np.float32)
